# revision 1
# baseline (speedup 1.0000x reference)
"""Cross-attention kernel for Trainium2, 8 NeuronCores.

Sharding (data + head parallel, per the problem's sharding hint):
  core c in 0..7 -> batch b = c // 4, head-pair hp = c % 4.
  Each core computes attention for its batch with 2 of the 8 heads
  (a 128-wide slice of the 512 hidden features), then the partial
  out-projection  attn_out_slice @ Wo[slice, :].  The host sums the 4
  partials per batch (the "all-reduce") ; bo is added on the hp==0 core.

Device-side dataflow per core (all matmuls fp32r, feature-major):
  qT[128, N] = Wq_sl.T @ x.T          (contraction over D=1024 in 8 chunks)
  kT[128, M] = Wk_sl.T @ ctx.T
  vT[128, M] = Wv_sl.T @ ctx.T
  V_aug[m,65] = PE-transpose of vT per head + ones column
  per n-chunk s (512 cols), per m-chunk mc (128 rows):
     St[m 128, n 1024] = [kT_h0_mc.T @ qT_h0_s | kT_h1_mc.T @ qT_h1_s]
         (two concurrent matmuls on PE row-groups 0-63 / 64-127)
     Pt = exp(St * 1/8)               (ScalarE, one op per m-chunk)
     Oaug_h[65, 512] += V_aug_h_mc.T @ Pt_h                (PSUM accum)
  row 64 of Oaug = softmax denominators; OT[h*64:, s] = Oaug[0:64]/denom
  out_p[n 128, 1024] = OT_ntile.T @ Wo_sl + bo             (per n-tile)
"""

import numpy as np

import concourse.bass as bass
import concourse.tile as tile
from concourse import bacc, mybir
from concourse.masks import make_identity

F32 = mybir.dt.float32
F32R = mybir.dt.float32r
BF16 = mybir.dt.bfloat16

USE_BF16 = True          # activation/weight dtype for all matmuls
PROBE_ST_F32R = False    # debug: keep the S^T matmul inputs fp32r
PROBE_PV_F32R = False    # debug: keep PV + outproj inputs fp32r
VPAD = 72                # PV weight row padded to 16B-aligned stride (bf16)
AT = BF16 if USE_BF16 else F32R

D = 1024      # model dim (contraction for projections)
SEQ = 2048    # n == m
F = 128       # features per core (2 heads x 64)
DH = 64       # head dim
NS = SEQ // 512   # 4 n-chunks of 512
NK = D // 128     # 8 contraction chunks
NM = SEQ // 128   # 16 m-chunks of 128
SCALE = DH ** -0.5


def build_nc():
    nc = bacc.Bacc("TRN2", target_bir_lowering=False, debug=False)

    xT_d = nc.dram_tensor("xT", [D, SEQ], AT, kind="ExternalInput")
    cT_d = nc.dram_tensor("cT", [D, SEQ], AT, kind="ExternalInput")
    # wq/wk/wv arrive pre-swizzled by the host: [128, NK*128] where
    # column block k holds W[k*128:(k+1)*128, :].T-chunk laid contiguously.
    wq_d = nc.dram_tensor("wq", [128, NK * 128], AT, kind="ExternalInput")
    wk_d = nc.dram_tensor("wk", [128, NK * 128], AT, kind="ExternalInput")
    wv_d = nc.dram_tensor("wv", [128, NK * 128], AT, kind="ExternalInput")
    WO_DT = F32R if PROBE_PV_F32R else AT
    wo_d = nc.dram_tensor("wo", [F, D], WO_DT, kind="ExternalInput")
    bo_d = nc.dram_tensor("bo", [1, D], F32, kind="ExternalInput")
    out_d = nc.dram_tensor("out_p", [SEQ, D], F32, kind="ExternalOutput")

    with tile.TileContext(nc) as tc:
        _emit(tc, nc, xT_d, cT_d, wq_d, wk_d, wv_d, wo_d, bo_d, out_d)
    nc.compile()
    return nc


def _emit(tc, nc, xT_d, cT_d, wq_d, wk_d, wv_d, wo_d, bo_d, out_d):
    from contextlib import ExitStack

    ctx = ExitStack()
    wpool = ctx.enter_context(tc.tile_pool(name="wpool", bufs=1))
    big = ctx.enter_context(tc.tile_pool(name="big", bufs=1))
    stream = ctx.enter_context(tc.tile_pool(name="stream", bufs=10))
    ptp = ctx.enter_context(tc.tile_pool(name="ptp", bufs=4))
    ostage = ctx.enter_context(tc.tile_pool(name="ostage", bufs=4))
    dscr = ctx.enter_context(tc.tile_pool(name="dscr", bufs=2, space="DRAM"))
    ps_small = ctx.enter_context(tc.tile_pool(name="ps_small", bufs=2, space="PSUM"))
    ps_st = ctx.enter_context(tc.tile_pool(name="ps_st", bufs=2, space="PSUM"))
    ps_oaug = ctx.enter_context(tc.tile_pool(name="ps_oaug", bufs=2, space="PSUM"))

    # ---- constants / weights (contiguous DMAs; host pre-swizzled) ----
    # Per-chunk DMA pieces so they spread across DMA queues (one dma_start
    # lands on a single queue at ~32 GB/s; splitting cuts arrival latency).
    def load_w(w_s, w_d):
        for k in range(NK):
            nc.sync.dma_start(
                out=w_s[:, k, :], in_=w_d.ap()[:, k * 128 : (k + 1) * 128]
            )

    wq_s = wpool.tile([128, NK, 128], AT, name="wq_s")
    wk_s = wpool.tile([128, NK, 128], AT, name="wk_s")
    wv_s = wpool.tile([128, NK, 128], AT, name="wv_s")
    load_w(wq_s, wq_d)
    wo_s = wpool.tile([128, D], F32R if PROBE_PV_F32R else AT, name="wo_s")
    bo_rep = wpool.tile([128, D], F32, name="bo_rep")
    ident = wpool.tile([128, DH], F32, name="ident")
    make_identity(nc, ident[0:DH, :])
    make_identity(nc, ident[DH:128, :])
    zbias = wpool.tile([128, 1], F32, name="zbias")
    nc.vector.memset(zbias, 0.0)

    ST_DT = F32R if PROBE_ST_F32R else AT
    qT = big.tile([128, SEQ], ST_DT, name="qT", tag="qT")
    kT = big.tile([128, SEQ], ST_DT, name="kT", tag="kT")
    vT = big.tile([128, SEQ], F32, name="vT", tag="vT")
    PV_DT = F32R if PROBE_PV_F32R else AT
    OT = big.tile([128, SEQ], PV_DT, name="OT", tag="OT")
    # V per head+m-chunk, with a ones column (65th) that accumulates the
    # softmax denominators during the PV matmul.
    Vall = big.tile([128, 2, NM, VPAD], PV_DT, name="Vall", tag="Vall")
    ones_sb = wpool.tile([128, 2 * NM], F32, name="ones_sb")
    nc.vector.memset(ones_sb, 1.0)
    nc.vector.tensor_copy(
        out=Vall[:, :, :, DH : DH + 1],
        in_=ones_sb.rearrange("p (h m o) -> p h m o", h=2, o=1),
    )
    if VPAD > DH + 1:
        zpad_sb = wpool.tile([128, 2 * NM * (VPAD - DH - 1)], F32, name="zpad_sb")
        nc.vector.memset(zpad_sb, 0.0)
        nc.vector.tensor_copy(
            out=Vall[:, :, :, DH + 1 :],
            in_=zpad_sb.rearrange("p (h m o) -> p h m o", h=2, m=NM),
        )

    def load_chunk(src_d, k, s):
        """[128, 512] activation chunk as 2 DMA pieces on separate queues."""
        chunk = stream.tile([128, 512], AT, name="chunk", tag="stream")
        for p in range(2):
            nc.sync.dma_start(
                out=chunk[:, p * 256 : (p + 1) * 256],
                in_=src_d.ap()[
                    k * 128 : (k + 1) * 128,
                    s * 512 + p * 256 : s * 512 + (p + 1) * 256,
                ],
            )
        return chunk

    def proj(dst, w_s, src_d, s):
        """dst[:, s*512:(s+1)*512] = w_s.T @ src_d (contraction over D)."""
        acc = ps_small.tile([128, 512], F32, name="proj_acc", tag="small")
        for k in range(NK):
            chunk = load_chunk(src_d, k, s)
            nc.tensor.matmul(
                acc, w_s[:, k, :], chunk,
                start=(k == 0), stop=(k == NK - 1),
            )
        nc.vector.tensor_copy(out=dst[:, s * 512 : (s + 1) * 512], in_=acc)

    def kv_proj(g):
        """k and v projections for seq chunk g (they share the ctx stream)."""
        kacc = ps_small.tile([128, 512], F32, name="kacc", tag="small")
        vacc = ps_small.tile([128, 512], F32, name="vacc", tag="small")
        for k in range(NK):
            chunk = load_chunk(cT_d, k, g)
            nc.tensor.matmul(
                kacc, wk_s[:, k, :], chunk,
                start=(k == 0), stop=(k == NK - 1),
            )
            nc.tensor.matmul(
                vacc, wv_s[:, k, :], chunk,
                start=(k == 0), stop=(k == NK - 1),
            )
        nc.vector.tensor_copy(out=kT[:, g * 512 : (g + 1) * 512], in_=kacc)
        nc.vector.tensor_copy(out=vT[:, g * 512 : (g + 1) * 512], in_=vacc)

    def v_transpose(g):
        """Vall[:, h, mc, 0:64] = vT[h*64:(h+1)*64, mc*128:(mc+1)*128].T"""
        for mc in range(4 * g, 4 * g + 4):
            for h in range(2):
                tp = ps_small.tile([128, DH], F32, name="tp", tag="small")
                nc.tensor.transpose(
                    tp,
                    vT[h * DH : (h + 1) * DH, mc * 128 : (mc + 1) * 128],
                    ident[h * DH : (h + 1) * DH, :],
                )
                nc.vector.tensor_copy(out=Vall[:, h, mc, 0:DH], in_=tp)

    def attn_group(s, oaug, mcs):
        """Attention for n-chunk s over the given m-chunks."""
        n0, n1 = s * 512, (s + 1) * 512
        for mc in mcs:
            m0, m1 = mc * 128, (mc + 1) * 128
            st = ps_st.tile([128, 1024], F32, name="st", tag="st")
            nc.tensor.matmul(
                st[:, 0:512], kT[0:DH, m0:m1], qT[0:DH, n0:n1],
                start=True, stop=True, tile_position=(0, 0),
            )
            nc.tensor.matmul(
                st[:, 512:1024], kT[DH:128, m0:m1], qT[DH:128, n0:n1],
                start=True, stop=True, tile_position=(64, 0),
            )
            pt = ptp.tile([128, 1024], PV_DT, name="pt", tag="pt")
            nc.scalar.activation(
                out=pt, in_=st,
                func=mybir.ActivationFunctionType.Exp,
                bias=zbias, scale=SCALE,
            )
            nc.tensor.matmul(
                oaug[0], Vall[:, 0, mc, 0 : DH + 1], pt[:, 0:512],
                start=(mc == 0), stop=(mc == NM - 1),
            )
            nc.tensor.matmul(
                oaug[1], Vall[:, 1, mc, 0 : DH + 1], pt[:, 512:1024],
                start=(mc == 0), stop=(mc == NM - 1),
            )

    def fin(s, oaug):
        """Normalize by softmax denominators (row 64 of oaug) into OT.

        The PSUM accumulators are evacuated to SBUF immediately so the
        banks free up for the next n-chunk's accumulation.  The [1, 512]
        denominator row is repartitioned to [128, 4] via a DRAM bounce so
        the reciprocal runs on all DVE lanes (a single-partition
        reciprocal measures ~3.3 us; this way it is ~30 ns + small DMAs).
        """
        n0, n1 = s * 512, (s + 1) * 512
        for h in range(2):
            oaug_sb = ostage.tile([DH + 1, 512], F32, name="oaug_sb",
                                  tag="oaug_sb", bufs=2)
            nc.vector.tensor_copy(out=oaug_sb, in_=oaug[h])
            den_p = ostage.tile([128, 4], F32, name="den_p", tag="den_p", bufs=2)
            nc.sync.dma_start(out=den_p, in_=oaug_sb[DH : DH + 1, :])
            rec_p = ostage.tile([128, 4], F32, name="rec_p", tag="rec_p", bufs=2)
            nc.vector.reciprocal(out=rec_p, in_=den_p)
            scr2 = dscr.tile([128, 4], F32, name="scr2", tag="scr2")
            nc.sync.dma_start(out=scr2, in_=rec_p)
            recip_rep = ostage.tile([DH, 512], F32, name="recip_rep",
                                    tag="recip_rep", bufs=2)
            nc.sync.dma_start(
                out=recip_rep,
                in_=scr2.rearrange("p f -> (p f)").partition_broadcast(DH),
            )
            nc.vector.tensor_mul(
                out=OT[h * DH : (h + 1) * DH, n0:n1],
                in0=oaug_sb[0:DH, :],
                in1=recip_rep,
            )

    def outproj(s):
        for t in range(4):
            nt = s * 4 + t
            for half in range(2):
                c0, c1 = half * 512, (half + 1) * 512
                ops = ps_small.tile([128, 512], F32, name="ops", tag="small")
                nc.tensor.matmul(
                    ops, OT[:, nt * 128 : (nt + 1) * 128], wo_s[:, c0:c1],
                    start=True, stop=True,
                )
                osb = ostage.tile([128, 512], F32, name="osb", tag="osb")
                nc.vector.tensor_add(out=osb, in0=ops, in1=bo_rep[:, c0:c1])
                nc.sync.dma_start(
                    out=out_d.ap()[nt * 128 : (nt + 1) * 128, c0:c1], in_=osb
                )

    # ---- schedule ----
    # q(s=0) first so attention over n-chunk 0 can start as soon as the
    # first k/v seq-chunk lands; kv groups stream in and attention(s=0)
    # chases them m-group by m-group.
    def mk_oaug(s):
        return [
            ps_oaug.tile([DH + 1, 512], F32, name=f"oaug{s}_{h}", tag="oaug")
            for h in range(2)
        ]

    proj(qT, wq_s, xT_d, 0)
    load_w(wk_s, wk_d)
    load_w(wv_s, wv_d)
    oaug_cur = mk_oaug(0)
    for g in range(NS):
        kv_proj(g)
        v_transpose(g)
        attn_group(0, oaug_cur, list(range(4 * g, 4 * g + 4)))
        if g == 0:
            # deferred so the big streaming DMAs win the early queue slots
            load_w(wo_s.rearrange("p (c f) -> p c f", c=NK), wo_d)
            nc.gpsimd.dma_start(
                out=bo_rep, in_=bo_d.ap()[0, :].partition_broadcast(128)
            )
    for s in range(1, NS):
        # next n-chunk's q projection first: it keeps PE busy while the
        # previous chunk's fin/outproj chain (small DMAs) resolves.
        proj(qT, wq_s, xT_d, s)
        oaug_next = mk_oaug(s)
        fin(s - 1, oaug_cur)
        attn_group(s, oaug_next, list(range(NM)))
        outproj(s - 1)
        oaug_cur = oaug_next
    fin(NS - 1, oaug_cur)
    outproj(NS - 1)

    ctx.close()


_NC = None


def _get_nc():
    global _NC
    if _NC is None:
        _NC = build_nc()
    return _NC


def _np_at():
    if USE_BF16:
        import ml_dtypes

        return ml_dtypes.bfloat16
    return np.float32


def _swizzle(w):
    """[1024, 128] -> [128, 8*128]: chunk k of the contraction dim lands in
    column block k, so the device DMA is fully contiguous."""
    return np.ascontiguousarray(
        np.asarray(w, np.float32).reshape(NK, 128, F).transpose(1, 0, 2)
        .reshape(128, NK * F).astype(_np_at())
    )


def shard_inputs(x, context, Wq, Wk, Wv, Wo, bo):
    x = np.asarray(x, np.float32)
    context = np.asarray(context, np.float32)
    Wq = np.asarray(Wq, np.float32)
    Wk = np.asarray(Wk, np.float32)
    Wv = np.asarray(Wv, np.float32)
    Wo = np.asarray(Wo, np.float32)
    bo = np.asarray(bo, np.float32)

    at = _np_at()
    xT = [np.ascontiguousarray(x[b].T).astype(at) for b in range(x.shape[0])]
    cT = [np.ascontiguousarray(context[b].T).astype(at) for b in range(context.shape[0])]
    zero_bo = np.zeros((1, D), np.float32)
    in_maps = []
    for c in range(8):
        b, hp = divmod(c, 4)
        f0 = hp * F
        in_maps.append(
            {
                "xT": xT[b],
                "cT": cT[b],
                "wq": _swizzle(Wq[:, f0 : f0 + F]),
                "wk": _swizzle(Wk[:, f0 : f0 + F]),
                "wv": _swizzle(Wv[:, f0 : f0 + F]),
                "wo": np.ascontiguousarray(Wo[f0 : f0 + F, :]).astype(
                    np.float32 if PROBE_PV_F32R else _np_at()
                ),
                "bo": bo.reshape(1, D) if hp == 0 else zero_bo,
            }
        )
    return in_maps


def kernel(x, context, Wq, Wk, Wv, Wo, bo):
    from concourse.bass_utils import run_bass_kernel_spmd

    in_maps = shard_inputs(x, context, Wq, Wk, Wv, Wo, bo)
    nc = _get_nc()
    res = run_bass_kernel_spmd(nc, in_maps, list(range(8)))
    out = np.zeros((2, SEQ, D), np.float32)
    for c in range(8):
        out[c // 4] += res.results[c]["out_p"]
    return out



# revision 3
# speedup vs baseline: 1.4752x; 1.4752x over previous
"""Cross-attention kernel for Trainium2, 8 NeuronCores.

Sharding (data + head parallel, per the problem's sharding hint):
  core c in 0..7 -> batch b = c // 4, head-pair hp = c % 4.
  Each core computes attention for its batch with 2 of the 8 heads
  (a 128-wide slice of the 512 hidden features), then the partial
  out-projection  attn_out_slice @ Wo[slice, :].  The host sums the 4
  partials per batch and adds bo (the "all-reduce" / unshard step).

Device-side dataflow per core (all matmul operands bf16):
  qT[128, N] = Wq_sl.T @ x.T          (contraction over D=1024 in 8 chunks)
  kT[128, M] = Wk_sl.T @ ctx.T
  vT[128, M] = Wv_sl.T @ ctx.T
  V_aug[m,65] = PE-transpose of vT per head + ones column
  per n-chunk s (512 cols), per m-chunk mc (128 rows):
     St[m 128, n 1024] = [kT_h0_mc.T @ qT_h0_s | kT_h1_mc.T @ qT_h1_s]
         (two concurrent matmuls on PE row-groups 0-63 / 64-127)
     Pt = exp(St * 1/8)               (ScalarE, one op per m-chunk)
     Oaug_h[65, 512] += V_aug_h_mc.T @ Pt_h                (PSUM accum)
  row 64 of Oaug = softmax denominators; OT[h*64:, s] = Oaug[0:64]/denom
  out_p[n 128, 1024] = OT_ntile.T @ Wo_sl  (bf16 partial, host adds bo)

Schedule notes (from trace analysis of the previous version):
  - Inputs are host-swizzled so each seq-chunk is ONE contiguous-line
    dma_start (128 x 8KB descriptors fan out over all 16 DMA engines);
    dma_start dispatch costs ~700ns serialized on its issuing engine,
    so few/large DMAs beat many/small ones.
  - The exp ACTIVATEs are the serial bottleneck (~86us on ScalarE);
    everything else (projections, PV, out-proj, DMAs) is interleaved
    into the attention slots so the ACT stream stays dense.
  - Dummy matmuls at t=0 warm the PE HAM clock gate (1.2 -> 2.4 GHz);
    a dummy exp preloads the ScalarE table set during the DMA head.
"""

import numpy as np

import concourse.bass as bass
import concourse.tile as tile
from concourse import bacc, mybir
from concourse.masks import make_identity

F32 = mybir.dt.float32
BF16 = mybir.dt.bfloat16

D = 1024      # model dim (contraction for projections)
SEQ = 2048    # n == m
F = 128       # features per core (2 heads x 64)
DH = 64       # head dim
NS = SEQ // 512   # 4 n-chunks of 512
NK = D // 128     # 8 contraction chunks
NM = SEQ // 128   # 16 m-chunks of 128
VPAD = 72         # PV weight row padded to 16B-aligned stride (bf16)
SCALE = DH ** -0.5
NWARM = 14        # PE warm-up dummy matmuls (~6us at 1.2GHz)


def build_nc():
    nc = bacc.Bacc("TRN2", target_bir_lowering=False, debug=False)

    # x / context arrive host-swizzled: row s*128+p, col k*512+j holds
    # xT[k*128+p, s*512+j], so the tile for seq-chunk s is one DMA with
    # 8KB contiguous lines.
    xs_d = nc.dram_tensor("xs", [NS * 128, NK * 512], BF16, kind="ExternalInput")
    cs_d = nc.dram_tensor("cs", [NS * 128, NK * 512], BF16, kind="ExternalInput")
    # wq/wk/wv pre-swizzled: [128, NK*128], column block k = W[k*128:(k+1)*128, :].T
    wq_d = nc.dram_tensor("wq", [128, NK * 128], BF16, kind="ExternalInput")
    wk_d = nc.dram_tensor("wk", [128, NK * 128], BF16, kind="ExternalInput")
    wv_d = nc.dram_tensor("wv", [128, NK * 128], BF16, kind="ExternalInput")
    wo_d = nc.dram_tensor("wo", [F, D], BF16, kind="ExternalInput")
    out_d = nc.dram_tensor("out_p", [SEQ, D], BF16, kind="ExternalOutput")

    with tile.TileContext(nc) as tc:
        _emit(tc, nc, xs_d, cs_d, wq_d, wk_d, wv_d, wo_d, out_d)
    nc.compile()
    return nc


def _emit(tc, nc, xs_d, cs_d, wq_d, wk_d, wv_d, wo_d, out_d):
    from contextlib import ExitStack

    ctx = ExitStack()
    wpool = ctx.enter_context(tc.tile_pool(name="wpool", bufs=1))
    big = ctx.enter_context(tc.tile_pool(name="big", bufs=1))
    ptp = ctx.enter_context(tc.tile_pool(name="ptp", bufs=4))
    ostage = ctx.enter_context(tc.tile_pool(name="ostage", bufs=2))
    opool = ctx.enter_context(tc.tile_pool(name="opool", bufs=4))
    dscr = ctx.enter_context(tc.tile_pool(name="dscr", bufs=2, space="DRAM"))
    ps_small = ctx.enter_context(tc.tile_pool(name="ps_small", bufs=2, space="PSUM"))
    ps_st = ctx.enter_context(tc.tile_pool(name="ps_st", bufs=2, space="PSUM"))
    ps_oaug = ctx.enter_context(tc.tile_pool(name="ps_oaug", bufs=2, space="PSUM"))

    # ---- constants ----
    ident = wpool.tile([128, DH], F32, name="ident")
    make_identity(nc, ident[0:DH, :])
    make_identity(nc, ident[DH:128, :])
    zbias = wpool.tile([128, 1], F32, name="zbias")
    nc.vector.memset(zbias, 0.0)
    junkw = wpool.tile([128, 128], BF16, name="junkw")
    nc.gpsimd.memset(junkw, 0.0)
    junkm = wpool.tile([128, 512], BF16, name="junkm")
    nc.gpsimd.memset(junkm, 0.0)

    # preload the exp table set on ScalarE while DMAs stream
    act_warm = wpool.tile([128, 1], F32, name="act_warm")
    nc.scalar.activation(
        out=act_warm, in_=zbias,
        func=mybir.ActivationFunctionType.Exp, bias=zbias, scale=1.0,
    )

    # ---- input DMA dispatches (order == arrival priority) ----
    xs_t = [big.tile([128, NK, 512], BF16, name=f"xs{s}", tag=f"xs{s}")
            for s in range(NS)]
    cs_t = [big.tile([128, NK, 512], BF16, name=f"cs{s}", tag=f"cs{s}")
            for s in range(NS)]

    def load_seq(t, d, s):
        nc.sync.dma_start(out=t[s], in_=d.ap()[s * 128:(s + 1) * 128, :])

    wq_s = wpool.tile([128, NK, 128], BF16, name="wq_s")
    wk_s = wpool.tile([128, NK, 128], BF16, name="wk_s")
    wv_s = wpool.tile([128, NK, 128], BF16, name="wv_s")
    wo_s = wpool.tile([128, D], BF16, name="wo_s")
    nc.gpsimd.dma_start(out=wk_s, in_=wk_d.ap())
    nc.gpsimd.dma_start(out=wv_s, in_=wv_d.ap())
    nc.gpsimd.dma_start(out=wq_s, in_=wq_d.ap())
    load_seq(cs_t, cs_d, 0)
    load_seq(xs_t, xs_d, 0)
    load_seq(cs_t, cs_d, 1)
    load_seq(cs_t, cs_d, 2)
    load_seq(cs_t, cs_d, 3)
    load_seq(xs_t, xs_d, 1)
    load_seq(xs_t, xs_d, 2)
    load_seq(xs_t, xs_d, 3)
    nc.gpsimd.dma_start(out=wo_s, in_=wo_d.ap())

    # ---- PE HAM warm-up (junk matmuls, no data deps) ----
    warm_ps = ps_small.tile([128, 512], F32, name="warm_ps", tag="small")
    for _ in range(NWARM):
        nc.tensor.matmul(warm_ps, junkw, junkm, start=True, stop=True)

    # ---- big SBUF tensors ----
    qT = big.tile([128, SEQ], BF16, name="qT", tag="qT")
    kT = big.tile([128, SEQ], BF16, name="kT", tag="kT")
    vT = big.tile([128, SEQ], F32, name="vT", tag="vT")
    OT = big.tile([128, SEQ], BF16, name="OT", tag="OT")
    # V per head+m-chunk, with a ones column (65th) that accumulates the
    # softmax denominators during the PV matmul.
    Vall = big.tile([128, 2, NM, VPAD], BF16, name="Vall", tag="Vall")
    nc.vector.memset(Vall, 0.0)
    ones_sb = wpool.tile([128, 2 * NM], F32, name="ones_sb")
    nc.vector.memset(ones_sb, 1.0)
    nc.vector.tensor_copy(
        out=Vall[:, :, :, DH:DH + 1],
        in_=ones_sb.rearrange("p (h m o) -> p h m o", h=2, o=1),
    )

    # ---- compute emitters ----
    def q_proj_mms(s, ks):
        """Partial q projection: chunks ks of the contraction accumulate."""
        nonlocal q_acc
        if ks[0] == 0:
            q_acc = ps_small.tile([128, 512], F32, name="q_acc", tag="small")
        for k in ks:
            nc.tensor.matmul(
                q_acc, wq_s[:, k, :], xs_t[s][:, k, :],
                start=(k == 0), stop=(k == NK - 1),
            )
        if ks[-1] == NK - 1:
            nc.vector.tensor_copy(out=qT[:, s * 512:(s + 1) * 512], in_=q_acc)

    def kv_proj_mms(g, ks):
        nonlocal k_acc, v_acc
        if ks[0] == 0:
            k_acc = ps_small.tile([128, 512], F32, name="k_acc", tag="small")
            v_acc = ps_small.tile([128, 512], F32, name="v_acc", tag="small")
        for k in ks:
            nc.tensor.matmul(
                k_acc, wk_s[:, k, :], cs_t[g][:, k, :],
                start=(k == 0), stop=(k == NK - 1),
            )
            nc.tensor.matmul(
                v_acc, wv_s[:, k, :], cs_t[g][:, k, :],
                start=(k == 0), stop=(k == NK - 1),
            )
        if ks[-1] == NK - 1:
            nc.vector.tensor_copy(out=kT[:, g * 512:(g + 1) * 512], in_=k_acc)
            nc.vector.tensor_copy(out=vT[:, g * 512:(g + 1) * 512], in_=v_acc)

    q_acc = k_acc = v_acc = None

    def v_transpose(g):
        """Vall[:, h, mc, 0:64] = vT[h*64:(h+1)*64, mc*128:(mc+1)*128].T"""
        for mc in range(4 * g, 4 * g + 4):
            for h in range(2):
                tp = ps_small.tile([128, DH], F32, name="tp", tag="small")
                nc.tensor.transpose(
                    tp,
                    vT[h * DH:(h + 1) * DH, mc * 128:(mc + 1) * 128],
                    ident[h * DH:(h + 1) * DH, :],
                )
                nc.vector.tensor_copy(out=Vall[:, h, mc, 0:DH], in_=tp)

    def st_mm(s, mc):
        n0, n1 = s * 512, (s + 1) * 512
        m0, m1 = mc * 128, (mc + 1) * 128
        st = ps_st.tile([128, 1024], F32, name="st", tag="st")
        nc.tensor.matmul(
            st[:, 0:512], kT[0:DH, m0:m1], qT[0:DH, n0:n1],
            start=True, stop=True, tile_position=(0, 0),
        )
        nc.tensor.matmul(
            st[:, 512:1024], kT[DH:128, m0:m1], qT[DH:128, n0:n1],
            start=True, stop=True, tile_position=(64, 0),
        )
        return st

    def act_exp(st):
        pt = ptp.tile([128, 1024], BF16, name="pt", tag="pt")
        nc.scalar.activation(
            out=pt, in_=st,
            func=mybir.ActivationFunctionType.Exp,
            bias=zbias, scale=SCALE,
        )
        return pt

    def pv_mm(oaug, mc, pt):
        nc.tensor.matmul(
            oaug[0], Vall[:, 0, mc, 0:DH + 1], pt[:, 0:512],
            start=(mc == 0), stop=(mc == NM - 1),
        )
        nc.tensor.matmul(
            oaug[1], Vall[:, 1, mc, 0:DH + 1], pt[:, 512:1024],
            start=(mc == 0), stop=(mc == NM - 1),
        )

    def mk_oaug(s):
        return [
            ps_oaug.tile([DH + 1, 512], F32, name=f"oaug{s}_{h}", tag="oaug")
            for h in range(2)
        ]

    def attn_s(s, fills):
        """One n-chunk of attention; fills[i] emits PE filler work that is
        injected between S^T(i+1) and PV(i) so it runs while ScalarE
        chews on exp."""
        oaug = mk_oaug(s)
        sts = [None, None]
        pts = [None, None]
        sts[0] = st_mm(s, 0)
        pts[0] = act_exp(sts[0])
        for mc in range(NM):
            # fills must be emitted BEFORE the S^T that reads what they
            # write (tile deps are built in emission order)
            if mc < len(fills) and fills[mc] is not None:
                fills[mc]()
            if mc < NM - 1:
                sts[(mc + 1) % 2] = st_mm(s, mc + 1)
                pts[(mc + 1) % 2] = act_exp(sts[(mc + 1) % 2])
            pv_mm(oaug, mc, pts[mc % 2])
        # eager PSUM evacuation: frees the oaug banks for the next n-chunk
        oaug_sb = []
        for h in range(2):
            t = ostage.tile([DH + 1, 512], F32, name="oaug_sb", tag="oaug_sb")
            nc.vector.tensor_copy(out=t, in_=oaug[h])
            oaug_sb.append(t)
        return oaug_sb

    def fin_rest(s, oaug_sb):
        """Normalize by softmax denominators (row 64 of oaug) into OT.

        The [1, 512] denominator row is repartitioned to [128, 4] via a
        DRAM bounce so the reciprocal runs on all DVE lanes.
        """
        n0, n1 = s * 512, (s + 1) * 512
        den_p, rec_p, rep = [], [], []
        for h in range(2):
            den_p.append(ostage.tile([128, 4], F32, name="den_p", tag="den_p"))
            nc.sync.dma_start(out=den_p[h], in_=oaug_sb[h][DH:DH + 1, :])
        for h in range(2):
            rec_p.append(ostage.tile([128, 4], F32, name="rec_p", tag="rec_p"))
            nc.vector.reciprocal(out=rec_p[h], in_=den_p[h])
        scr = []
        for h in range(2):
            scr.append(dscr.tile([128, 4], F32, name="scr", tag="scr"))
            nc.sync.dma_start(out=scr[h], in_=rec_p[h])
        for h in range(2):
            rep.append(ostage.tile([DH, 512], F32, name="rep", tag="rep"))
            nc.sync.dma_start(
                out=rep[h],
                in_=scr[h].rearrange("p f -> (p f)").partition_broadcast(DH),
            )
        for h in range(2):
            nc.vector.tensor_mul(
                out=OT[h * DH:(h + 1) * DH, n0:n1],
                in0=oaug_sb[h][0:DH, :],
                in1=rep[h],
            )

    def outproj_tile(s, t, n_dma_pieces=1):
        nt = s * 4 + t
        osb = opool.tile([128, 1024], BF16, name="osb", tag="osb")
        for half in range(2):
            c0, c1 = half * 512, (half + 1) * 512
            ops = ps_small.tile([128, 512], F32, name="ops", tag="small")
            nc.tensor.matmul(
                ops, OT[:, nt * 128:(nt + 1) * 128], wo_s[:, c0:c1],
                start=True, stop=True,
            )
            nc.vector.tensor_copy(out=osb[:, c0:c1], in_=ops)
        for p in range(n_dma_pieces):
            w = 1024 // n_dma_pieces
            nc.sync.dma_start(
                out=out_d.ap()[nt * 128:(nt + 1) * 128, p * w:(p + 1) * w],
                in_=osb[:, p * w:(p + 1) * w],
            )

    # ---- schedule ----
    kv_proj_mms(0, list(range(NK)))
    v_transpose(0)
    q_proj_mms(0, list(range(NK)))

    # s=0 fills: chase the kv projections of groups 1..3, then q(1).
    fills0 = [None] * NM
    fills0[1] = lambda: kv_proj_mms(1, [0, 1, 2])
    fills0[2] = lambda: kv_proj_mms(1, [3, 4, 5])
    fills0[3] = lambda: kv_proj_mms(1, [6, 7])
    fills0[4] = lambda: v_transpose(1)
    fills0[5] = lambda: kv_proj_mms(2, [0, 1, 2])
    fills0[6] = lambda: kv_proj_mms(2, [3, 4, 5])
    fills0[7] = lambda: kv_proj_mms(2, [6, 7])
    fills0[8] = lambda: v_transpose(2)
    fills0[9] = lambda: kv_proj_mms(3, [0, 1, 2])
    fills0[10] = lambda: kv_proj_mms(3, [3, 4, 5])
    fills0[11] = lambda: kv_proj_mms(3, [6, 7])
    fills0[12] = lambda: v_transpose(3)
    fills0[13] = lambda: q_proj_mms(1, [0, 1, 2, 3])
    fills0[14] = lambda: q_proj_mms(1, [4, 5, 6, 7])

    def mk_fills(qs, op_s):
        """Fills for attn chunk s>=1: q projection of chunk qs early,
        out-projection of chunk op_s late (after its fin completes)."""
        f = [None] * NM
        if qs is not None:
            f[1] = lambda: q_proj_mms(qs, [0, 1, 2, 3])
            f[2] = lambda: q_proj_mms(qs, [4, 5, 6, 7])
        if op_s is not None:
            for i, t in enumerate(range(4)):
                f[10 + i] = (lambda tt: lambda: outproj_tile(op_s, tt))(t)
        return f

    oaug_sb = attn_s(0, fills0)
    fin_rest(0, oaug_sb)
    oaug_sb = attn_s(1, mk_fills(2, 0))
    fin_rest(1, oaug_sb)
    oaug_sb = attn_s(2, mk_fills(3, 1))
    fin_rest(2, oaug_sb)
    oaug_sb = attn_s(3, mk_fills(None, 2))
    fin_rest(3, oaug_sb)
    for t in range(4):
        outproj_tile(3, t, n_dma_pieces=2)

    ctx.close()


_NC = None


def _get_nc():
    global _NC
    if _NC is None:
        _NC = build_nc()
    return _NC


def _bf16():
    import ml_dtypes

    return ml_dtypes.bfloat16


def _swizzle_w(w):
    """[1024, 128] -> [128, 8*128]: chunk k of the contraction dim lands in
    column block k, so the device DMA is fully contiguous."""
    return np.ascontiguousarray(
        np.asarray(w, np.float32).reshape(NK, 128, F).transpose(1, 0, 2)
        .reshape(128, NK * F).astype(_bf16())
    )


def _swizzle_act(aT):
    """[1024, 2048] -> [512, 4096] bf16 with row s*128+p, col k*512+j =
    aT[k*128+p, s*512+j]: seq-chunk s is rows [s*128, (s+1)*128) with
    fully contiguous 8KB lines."""
    return np.ascontiguousarray(
        aT.reshape(NK, 128, NS, 512).transpose(2, 1, 0, 3)
        .reshape(NS * 128, NK * 512).astype(_bf16())
    )


def shard_inputs(x, context, Wq, Wk, Wv, Wo, bo):
    x = np.asarray(x, np.float32)
    context = np.asarray(context, np.float32)
    Wq = np.asarray(Wq, np.float32)
    Wk = np.asarray(Wk, np.float32)
    Wv = np.asarray(Wv, np.float32)
    Wo = np.asarray(Wo, np.float32)

    xs = [_swizzle_act(np.ascontiguousarray(x[b].T)) for b in range(x.shape[0])]
    cs = [_swizzle_act(np.ascontiguousarray(context[b].T))
          for b in range(context.shape[0])]
    in_maps = []
    for c in range(8):
        b, hp = divmod(c, 4)
        f0 = hp * F
        in_maps.append(
            {
                "xs": xs[b],
                "cs": cs[b],
                "wq": _swizzle_w(Wq[:, f0:f0 + F]),
                "wk": _swizzle_w(Wk[:, f0:f0 + F]),
                "wv": _swizzle_w(Wv[:, f0:f0 + F]),
                "wo": np.ascontiguousarray(Wo[f0:f0 + F, :]).astype(_bf16()),
            }
        )
    return in_maps


def kernel(x, context, Wq, Wk, Wv, Wo, bo):
    from concourse.bass_utils import run_bass_kernel_spmd

    in_maps = shard_inputs(x, context, Wq, Wk, Wv, Wo, bo)
    nc = _get_nc()
    res = run_bass_kernel_spmd(nc, in_maps, list(range(8)))
    out = np.zeros((2, SEQ, D), np.float32)
    for c in range(8):
        out[c // 4] += np.asarray(res.results[c]["out_p"], np.float32)
    out += np.asarray(bo, np.float32).reshape(1, 1, D)
    return out
